# revision 1
# baseline (speedup 1.0000x reference)
"""MatchNet retrieval-KNN kernel for 8 Trainium2 NeuronCores.

Strategy (candidate-sharded, bf16 device pass + exact fp32 host re-score):
  - Host (fp32 BLAS): A = W^T W; xat = A @ x^T; Z = C @ A;
    cn2 = rowsum(C * Z) = c^T A c;  g = -cn2/2, mean-centered (constant
    per-row shifts don't change ranking) and cast to bf16.
  - Device (per core), all-bf16 matmuls at full PE rate:
      score[q,n] = (A x_q).c_n + g'_n   == s' + const  (monotone in -dist^2)
    computed as 2 K-tile matmuls + one K=1 bias matmul per [128q x 512n]
    PSUM tile; ACT copies each tile to SBUF bf16; DVE max/max_index extract
    top-8 values+indices per 256-candidate segment -> [1024, 400] noisy
    pool per core (device noise ~0.3 absolute).
  - Host: merge the 8 cores' pools (3200 noisy candidates/row), take
    top-K_SAFE by noisy value (true top-32 is inside: noise << the
    rank-32..K_SAFE exact-score gap), re-score those exactly in fp32 via
    s' = (x A).c - cn2/2 (reusing Z), take exact top-32, softmax(-dist),
    weighted-sum candidate_y.  Verified vs the fp32 reference.

Toolchain note: walrus here rejects >1 sync wait per instruction;
_legalize_waits() peels extra waits onto single-wait same-engine NoOps in
the BIR JSON (engines execute their stream in order, so blocking is
equivalent).
"""

import json
import os
import types

import ml_dtypes
import numpy as np

import concourse.bass as bass
import concourse.mybir as mybir
import concourse.tile as tile
from concourse.bass import ds
from concourse.bass_utils import run_bass_kernel_spmd

B, N, D_IN, DIM, NUMK = 1024, 100000, 256, 512, 32
TEMP = 1.0
NCORES = 8
NSHARD_REAL = N // NCORES  # 12500
NSHARD = 12800             # padded per-core candidate count
CHUNK = 2560
NCHUNKS = NSHARD // CHUNK  # 5
NT = CHUNK // 512          # psum tiles per chunk (5)
SEG = 256
SEGS = NSHARD // SEG       # 50
U_W = SEGS * 8             # 400 candidate slots per row per core
QT = B // 128              # 8 query tiles
KD = D_IN // 128           # 2 contraction tiles
XA_W = B + 136             # xat | ones-row(128) | pad
K_SAFE = 64                # host re-scores this many noisy-top per row

F32 = mybir.dt.float32
BF16 = mybir.dt.bfloat16
ACT_COPY = mybir.ActivationFunctionType.Copy


def _legalize_waits(nc):
    """Wrap nc.to_json_bytes so every instruction carries <=1 sync wait."""
    orig = nc.to_json_bytes

    def patched(self):
        m = json.loads(orig())
        ctr = 0
        for fn in m["functions"]:
            for blk in fn["blocks"]:
                out = []
                for inst in blk["instructions"]:
                    si = inst.get("sync_info")
                    waits = (si or {}).get("on_wait") or []
                    if len(waits) > 1:
                        for w in waits[:-1]:
                            ctr += 1
                            out.append({
                                "debug": inst.get("debug", 0),
                                "engine": inst["engine"],
                                "ins": [],
                                "name": f"I-nopw{ctr}",
                                "opcode": "NoOp",
                                "outs": [],
                                "sync_info": {"on_wait": [w],
                                              "on_update": []},
                            })
                        si["on_wait"] = waits[-1:]
                    out.append(inst)
                blk["instructions"] = out
        return json.dumps(m).encode()

    nc.to_json_bytes = types.MethodType(patched, nc)
    return nc


def _build_bass():
    nc = bass.Bass()
    xa_d = nc.dram_tensor("xa", [D_IN, XA_W], BF16, kind="ExternalInput")
    cxt_d = nc.dram_tensor("cxt", [D_IN, NSHARD], BF16, kind="ExternalInput")
    gb_d = nc.dram_tensor("gb", [1, NSHARD], BF16, kind="ExternalInput")
    oval_d = nc.dram_tensor("out_val", [B, U_W], BF16, kind="ExternalOutput")
    oidx_d = nc.dram_tensor("out_idx", [B, U_W], mybir.dt.uint32,
                            kind="ExternalOutput")

    with (
        tile.TileContext(nc) as tc,
        tc.tile_pool(name="const", bufs=1) as constp,
        tc.tile_pool(name="cx", bufs=NCHUNKS) as cxp,
        tc.tile_pool(name="s", bufs=6) as sp,
        tc.tile_pool(name="sps", bufs=7, space="PSUM") as spsp,
    ):
        xa_sb = constp.tile([128, KD, XA_W], BF16)
        nc.sync.dma_start(
            xa_sb, xa_d.rearrange("(ko ki) q -> ki ko q", ki=128))
        gb_sb = constp.tile([1, NSHARD], BF16)
        nc.sync.dma_start(gb_sb, gb_d[:, :])
        # host fills xa[0, B : B+128] = 1
        ones_row = xa_sb[0:1, 0, ds(B, 128)]

        uval_all = constp.tile([128, QT, U_W], BF16, name="uval_all")
        uidx_all = constp.tile([128, QT, U_W], mybir.dt.uint32,
                               name="uidx_all")

        def xat(k, q):   # lhsT tile of A @ x^T
            return xa_sb[:, k, ds(q * 128, 128)]

        for c in range(NCHUNKS):
            cx_sb = cxp.tile([128, KD, CHUNK], BF16)
            nc.sync.dma_start(
                cx_sb,
                cxt_d[:, ds(c * CHUNK, CHUNK)].rearrange(
                    "(ko ki) n -> ki ko n", ki=128))

            for q in range(QT):
                for nt in range(NT):
                    sps = spsp.tile([128, 512], F32)
                    for k in range(KD):
                        nc.tensor.matmul(
                            sps,
                            xat(k, q),
                            cx_sb[:, k, ds(nt * 512, 512)],
                            start=(k == 0), stop=False)
                    nc.tensor.matmul(
                        sps, ones_row,
                        gb_sb[:, ds(c * CHUNK + nt * 512, 512)],
                        start=False, stop=True)
                    s_sb = sp.tile([128, 512], BF16, name="s_sb")
                    nc.scalar.activation(s_sb, sps, ACT_COPY)
                    for si in range(512 // SEG):
                        slot = ((c * NT + nt) * (512 // SEG) + si) * 8
                        nc.vector.max(
                            out=uval_all[:, q, ds(slot, 8)],
                            in_=s_sb[:, ds(si * SEG, SEG)])
                        nc.vector.max_index(
                            out=uidx_all[:, q, ds(slot, 8)],
                            in_max=uval_all[:, q, ds(slot, 8)],
                            in_values=s_sb[:, ds(si * SEG, SEG)])

        nc.gpsimd.dma_start(
            oval_d.rearrange("(q p) w -> p q w", p=128), uval_all)
        nc.gpsimd.dma_start(
            oidx_d.rearrange("(q p) w -> p q w", p=128), uidx_all)
    return _legalize_waits(nc)


_NC_CACHE = {}


def kernel(x, candidate_x, candidate_y, W, b, context_size, is_train):
    x = np.asarray(x, dtype=np.float32)
    candidate_x = np.asarray(candidate_x, dtype=np.float32)
    candidate_y = np.asarray(candidate_y, dtype=np.float32)
    W = np.asarray(W, dtype=np.float32)
    b = np.asarray(b, dtype=np.float32)

    A = (W.T @ W).astype(np.float32)           # [256, 256]
    xat = (A @ x.T).astype(np.float32)         # [256, 1024]
    Z = candidate_x @ A                        # [N, 256]  (reused in rescore)
    cn2 = np.einsum("ij,ij->i", candidate_x, Z)  # c^T A c
    g = -0.5 * cn2
    gmean = float(g.mean())
    gc = (g - gmean).astype(np.float32)        # centered: ranking unchanged

    ones_blk = np.zeros((D_IN, 136), dtype=np.float32)
    ones_blk[0, :128] = 1.0
    xa_bf = np.concatenate([xat, ones_blk], axis=1).astype(ml_dtypes.bfloat16)

    cxt_full = np.ascontiguousarray(candidate_x.T)  # [256, 100000]
    in_maps = []
    for c in range(NCORES):
        cxt = np.zeros((D_IN, NSHARD), dtype=np.float32)
        cxt[:, :NSHARD_REAL] = cxt_full[:, c * NSHARD_REAL:(c + 1) * NSHARD_REAL]
        gb = np.full((1, NSHARD), -1e30, dtype=np.float32)  # pads lose
        gb[0, :NSHARD_REAL] = gc[c * NSHARD_REAL:(c + 1) * NSHARD_REAL]
        in_maps.append({"xa": xa_bf,
                        "cxt": cxt.astype(ml_dtypes.bfloat16),
                        "gb": gb.astype(ml_dtypes.bfloat16)})

    if "nc" not in _NC_CACHE:
        _NC_CACHE["nc"] = _build_bass()
    nc = _NC_CACHE["nc"]

    trace = bool(int(os.environ.get("KERNEL_TRACE", "0")))
    res = run_bass_kernel_spmd(nc, in_maps, core_ids=list(range(NCORES)),
                               trace=trace)
    if trace:
        print(f"HW exec time: {res.exec_time_ns} ns")
        print(f"mean exec time: {res.mean_exec_time_ns} ns")
        if res.instructions_and_trace is not None:
            print("trace:", res.instructions_and_trace[1])

    # ---- host merge: noisy top-K_SAFE, then exact fp32 re-score ----
    vals = np.concatenate(
        [r["out_val"].astype(np.float32) for r in res.results], axis=1)
    idxs = np.concatenate(
        [(np.arange(U_W, dtype=np.int64) // 8 * SEG)[None, :]
         + r["out_idx"].astype(np.int64)
         + c * NSHARD_REAL
         for c, r in enumerate(res.results)], axis=1)          # [B, 3200]

    sel = np.argpartition(-vals, K_SAFE, axis=1)[:, :K_SAFE]
    rows = np.arange(B)[:, None]
    cand = np.clip(idxs[rows, sel], 0, N - 1)                  # [B, K_SAFE]

    # exact s' for the surviving candidates (Z, cn2 already computed)
    xA = xat.T                                                 # [1024, 256]
    C_sel = candidate_x[cand]                                  # [B, K, 256]
    s_exact = (np.einsum("rd,rkd->rk", xA, C_sel, dtype=np.float64)
               - 0.5 * cn2[cand])                              # [B, K]

    ordK = np.argsort(-s_exact, axis=1, kind="stable")
    top = ordK[:, :NUMK]
    s_sel = s_exact[rows, top]
    cand_sel = cand[rows, top]

    xe = (x @ W.T + b).astype(np.float32)
    xn2 = np.sum(xe.astype(np.float64) ** 2, axis=1)
    const_q = x.astype(np.float64) @ (W.T @ b).astype(np.float64) \
        + 0.5 * float(b.astype(np.float64) @ b.astype(np.float64))

    d2 = xn2[:, None] - 2.0 * (s_sel + const_q[:, None])
    d = np.sqrt(np.maximum(d2, 0.0)) / TEMP
    neg = -d
    neg -= neg.max(axis=1, keepdims=True)
    w = np.exp(neg)
    w /= w.sum(axis=1, keepdims=True)
    logits = np.sum(w * candidate_y[cand_sel].astype(np.float64), axis=1)

    # Rows whose rank-32/33 gap is within fp32 rounding ambiguity: re-rank
    # with reference-style fp32 arithmetic so the boundary pick matches.
    gap = (s_exact[rows[:, 0], ordK[:, NUMK - 1]]
           - s_exact[rows[:, 0], ordK[:, NUMK]])
    for r in np.where(gap < 0.01)[0]:
        csel = cand[r]
        ce_sel = (candidate_x[csel] @ W.T + b).astype(np.float32)
        sq = (np.sum(xe[r] ** 2, dtype=np.float32)
              + np.sum(ce_sel ** 2, axis=1, dtype=np.float32)
              - 2.0 * (ce_sel @ xe[r]))
        d_r = np.sqrt(np.maximum(sq, 0.0)) / TEMP
        o32 = np.argsort(d_r, kind="stable")[:NUMK]
        nb = (-d_r[o32]).astype(np.float64)
        nb -= nb.max()
        wr = np.exp(nb)
        wr /= wr.sum()
        logits[r] = float(wr @ candidate_y[csel[o32]].astype(np.float64))
    return logits.astype(np.float32)



# revision 4
# speedup vs baseline: 2.7334x; 2.7334x over previous
"""MatchNet retrieval-KNN kernel for 8 Trainium2 NeuronCores.

Strategy (candidate-sharded, fp8 DoubleRow device screen + exact fp32 host
re-score):
  - Host: A = W^T W = V diag(lam) V^T.  Query rows q_i = sqrt(lam_i) (V^T x)_i,
    candidate rows r_i = sqrt(lam_i) (V^T c)_i, so s = (Ax).c = q.r.  The
    ranking-relevant bias g = -||c||_A^2/2 (mean-centered) is folded into the
    256-dim contraction by dropping the weakest eigendirection (lam_min ~ 0.11,
    error ~0.1 absolute, far below the fp8 screen noise) and using that slot as
    (alpha, g/alpha).  Both operands quantized to fp8 e4m3.
  - Device (per core, 12800-candidate shard): one DoubleRow fp8 matmul per
    [128q x 512c] PSUM tile (K=256 in one instruction, 0.5 cyc/row), then
    window-32 max-pooling of the scores:
      * "direct" chunks: DVE pool_max straight from PSUM fp32,
      * "drain" chunks: ACT copies PSUM->SBUF bf16, DVE pools the bf16 chunk,
    balancing the two engines.  Output: 400 bf16 window maxima per query per
    core - window position IS the candidate index, no index extraction needed.
  - Host: merge 8x400 window maxima per row, take the top W_SEL=64 windows,
    exactly re-score those 64*32 candidates in fp32 (s = (xA).c - cn2/2), take
    exact top-32, softmax(-dist), weighted-sum candidate_y.  Rows whose
    rank-32/33 gap is within fp32 rounding ambiguity are re-ranked with
    reference-style fp32 arithmetic.

Toolchain note: walrus here rejects >1 sync wait per instruction;
_legalize_waits() peels extra waits onto single-wait same-engine NoOps in
the BIR JSON (engines execute their stream in order, so blocking is
equivalent).
"""

import json
import os
import types

import ml_dtypes
import numpy as np

import concourse.bass as bass
import concourse.mybir as mybir
import concourse.tile as tile
from concourse.bass import ds
from concourse.bass_utils import run_bass_kernel_spmd

B, N, D_IN, DIM, NUMK = 1024, 100000, 256, 512, 32
TEMP = 1.0
NCORES = 8
NSHARD_REAL = N // NCORES  # 12500
NSHARD = 12800             # padded per-core candidate count
QT = B // 128              # 8 query tiles
NT = NSHARD // 512         # 25 candidate tiles per query tile
WIN = 32
NWIN = NSHARD // WIN       # 400 windows per row per core
CHUNK_T = 5                # tiles per chunk (for drain-path grouping)
NCHUNK = NT // CHUNK_T     # 5 chunks per qtile
DRAIN_CHUNKS = (0, 2, 4)   # chunks drained via ACT->bf16 (rest: DVE direct)
W_SEL = 64                 # host re-scores this many windows per row

F32 = mybir.dt.float32
BF16 = mybir.dt.bfloat16
FP8 = mybir.dt.float8e4
ACT_COPY = mybir.ActivationFunctionType.Copy
DR = mybir.MatmulPerfMode.DoubleRow


def _legalize_waits(nc):
    """Wrap nc.to_json_bytes so every instruction carries <=1 sync wait."""
    orig = nc.to_json_bytes

    def patched(self):
        m = json.loads(orig())
        ctr = 0
        for fn in m["functions"]:
            for blk in fn["blocks"]:
                out = []
                for inst in blk["instructions"]:
                    si = inst.get("sync_info")
                    waits = (si or {}).get("on_wait") or []
                    if len(waits) > 1:
                        for w in waits[:-1]:
                            ctr += 1
                            out.append({
                                "debug": inst.get("debug", 0),
                                "engine": inst["engine"],
                                "ins": [],
                                "name": f"I-nopw{ctr}",
                                "opcode": "NoOp",
                                "outs": [],
                                "sync_info": {"on_wait": [w],
                                              "on_update": []},
                            })
                        si["on_wait"] = waits[-1:]
                    out.append(inst)
                blk["instructions"] = out
        return json.dumps(m).encode()

    nc.to_json_bytes = types.MethodType(patched, nc)
    return nc


def _build_bass():
    nc = bass.Bass()
    xa_d = nc.dram_tensor("xa", [D_IN, B], FP8, kind="ExternalInput")
    cxt_d = nc.dram_tensor("cxt", [D_IN, NSHARD], FP8, kind="ExternalInput")
    oval_d = nc.dram_tensor("out_val", [B, NWIN], BF16,
                            kind="ExternalOutput")

    with (
        tile.TileContext(nc) as tc,
        tc.tile_pool(name="const", bufs=1) as constp,
        tc.tile_pool(name="s", bufs=3) as sp,
        tc.tile_pool(name="sps", bufs=8, space="PSUM") as spsp,
    ):
        xa_sb = constp.tile([128, 2, B], FP8)
        nc.sync.dma_start(
            xa_sb, xa_d.rearrange("(ko ki) q -> ki ko q", ki=128))
        cx_sb = constp.tile([128, 2, NSHARD], FP8)
        nc.sync.dma_start(
            cx_sb, cxt_d.rearrange("(ko ki) n -> ki ko n", ki=128))

        uval_all = constp.tile([128, QT, NWIN], BF16, name="uval_all")

        for q in range(QT):
            for c in range(NCHUNK):
                if c in DRAIN_CHUNKS:
                    s_sb = sp.tile([128, CHUNK_T * 16, WIN], BF16,
                                   name="s_sb")
                    for t in range(CHUNK_T):
                        nt = c * CHUNK_T + t
                        sps = spsp.tile([128, 16, WIN], F32)
                        nc.tensor.matmul(
                            sps,
                            xa_sb[:, :, ds(q * 128, 128)],
                            cx_sb[:, :, ds(nt * 512, 512)],
                            start=True, stop=True, perf_mode=DR)
                        nc.scalar.activation(
                            s_sb[:, ds(t * 16, 16), :], sps, ACT_COPY)
                    nc.vector.tensor_reduce(
                        uval_all[:, q, ds(c * CHUNK_T * 16, CHUNK_T * 16)],
                        s_sb, axis=mybir.AxisListType.X,
                        op=mybir.AluOpType.max)
                else:
                    for t in range(CHUNK_T):
                        nt = c * CHUNK_T + t
                        sps = spsp.tile([128, 16, WIN], F32)
                        nc.tensor.matmul(
                            sps,
                            xa_sb[:, :, ds(q * 128, 128)],
                            cx_sb[:, :, ds(nt * 512, 512)],
                            start=True, stop=True, perf_mode=DR)
                        nc.vector.tensor_reduce(
                            uval_all[:, q, ds(nt * 16, 16)], sps,
                            axis=mybir.AxisListType.X,
                            op=mybir.AluOpType.max)

        nc.gpsimd.dma_start(
            oval_d.rearrange("(q p) w -> p q w", p=128), uval_all)
    return _legalize_waits(nc)


_NC_CACHE = {}


def kernel(x, candidate_x, candidate_y, W, b, context_size, is_train):
    x = np.asarray(x, dtype=np.float32)
    candidate_x = np.asarray(candidate_x, dtype=np.float32)
    candidate_y = np.asarray(candidate_y, dtype=np.float32)
    W = np.asarray(W, dtype=np.float32)
    b = np.asarray(b, dtype=np.float32)

    A = (W.T @ W).astype(np.float64)           # [256, 256]
    lam, V = np.linalg.eigh(A)                 # ascending eigenvalues
    sq = np.sqrt(lam[1:]).astype(np.float32)   # drop weakest eigendirection
    Vk = V[:, 1:].astype(np.float32)           # [256, 255]

    Qm = (Vk * sq).T @ x.T                     # [255, B]
    Rm = (Vk * sq).T @ candidate_x.T           # [255, N]
    p0 = candidate_x @ V[:, 0].astype(np.float32)
    cn2 = np.sum(Rm.astype(np.float64) ** 2, axis=0) \
        + lam[0] * p0.astype(np.float64) ** 2  # c^T A c
    g = (-0.5 * cn2).astype(np.float32)
    gc = g - g.mean()                          # centered: ranking unchanged
    alpha = float(2.0 ** round(np.log2(np.sqrt(gc.std()))))

    xa = np.empty((D_IN, B), dtype=np.float32)
    xa[0, :] = alpha
    xa[1:, :] = Qm
    xa_f8 = xa.astype(ml_dtypes.float8_e4m3)

    in_maps = []
    for c in range(NCORES):
        cxt = np.zeros((D_IN, NSHARD), dtype=np.float32)
        sl = slice(c * NSHARD_REAL, (c + 1) * NSHARD_REAL)
        cxt[0, :NSHARD_REAL] = gc[sl] / alpha
        cxt[0, NSHARD_REAL:] = -240.0          # pads lose every window
        cxt[1:, :NSHARD_REAL] = Rm[:, sl]
        in_maps.append({"xa": xa_f8,
                        "cxt": cxt.astype(ml_dtypes.float8_e4m3)})

    if "nc" not in _NC_CACHE:
        _NC_CACHE["nc"] = _build_bass()
    nc = _NC_CACHE["nc"]

    trace = bool(int(os.environ.get("KERNEL_TRACE", "0")))
    res = run_bass_kernel_spmd(nc, in_maps, core_ids=list(range(NCORES)),
                               trace=trace)
    if trace:
        print(f"HW exec time: {res.exec_time_ns} ns")
        print(f"mean exec time: {res.mean_exec_time_ns} ns")
        if res.instructions_and_trace is not None:
            print("trace:", res.instructions_and_trace[1])

    # ---- host: merge window maxima, exact fp32 re-score of top windows ----
    pooled = np.concatenate(
        [r["out_val"].astype(np.float32) for r in res.results], axis=1)
    # window id -> global candidate base
    wcore = np.arange(NCORES * NWIN) // NWIN
    wwin = np.arange(NCORES * NWIN) % NWIN
    wbase = wcore * NSHARD_REAL + wwin * WIN   # [3200]

    rows = np.arange(B)[:, None]
    selw = np.argpartition(-pooled, W_SEL, axis=1)[:, :W_SEL]  # [B, 64]
    cand = (wbase[selw][:, :, None]
            + np.arange(WIN)[None, None, :]).reshape(B, -1)    # [B, 2048]
    valid = cand < ((cand // NSHARD_REAL) * NSHARD_REAL + NSHARD_REAL)
    # windows at a shard tail may spill past the real candidates
    valid &= cand < N
    cand_c = np.minimum(cand, N - 1)

    xA = (A.astype(np.float32).T @ x.T).T      # [B, 256] = x A
    C_sel = candidate_x[cand_c]                # [B, 2048, 256]
    s_sel = np.einsum("rd,rkd->rk", xA, C_sel,
                      dtype=np.float32, casting="same_kind")
    s_sel = s_sel.astype(np.float64) - 0.5 * cn2[cand_c]
    s_sel[~valid] = -1e30

    ordK = np.argsort(-s_sel, axis=1, kind="stable")
    top = ordK[:, :NUMK]
    s_top = np.take_along_axis(s_sel, top, axis=1)
    cand_top = np.take_along_axis(cand_c, top, axis=1)

    xe = (x @ W.T + b).astype(np.float32)
    xn2 = np.sum(xe.astype(np.float64) ** 2, axis=1)
    const_q = x.astype(np.float64) @ (W.T @ b).astype(np.float64) \
        + 0.5 * float(b.astype(np.float64) @ b.astype(np.float64))

    d2 = xn2[:, None] - 2.0 * (s_top + const_q[:, None])
    d = np.sqrt(np.maximum(d2, 0.0)) / TEMP
    neg = -d
    neg -= neg.max(axis=1, keepdims=True)
    w = np.exp(neg)
    w /= w.sum(axis=1, keepdims=True)
    logits = np.sum(w * candidate_y[cand_top].astype(np.float64), axis=1)

    # Rows whose rank-32/33 gap is within fp32 rounding ambiguity: re-rank
    # with reference-style fp32 arithmetic so the boundary pick matches.
    gap = (s_sel[rows[:, 0], ordK[:, NUMK - 1]]
           - s_sel[rows[:, 0], ordK[:, NUMK]])
    for r in np.where(gap < 0.01)[0]:
        csel = cand_c[r][valid[r]]
        ce_sel = (candidate_x[csel] @ W.T + b).astype(np.float32)
        sq_r = (np.sum(xe[r] ** 2, dtype=np.float32)
                + np.sum(ce_sel ** 2, axis=1, dtype=np.float32)
                - 2.0 * (ce_sel @ xe[r]))
        d_r = np.sqrt(np.maximum(sq_r, 0.0)) / TEMP
        o32 = np.argsort(d_r, kind="stable")[:NUMK]
        nb = (-d_r[o32]).astype(np.float64)
        nb -= nb.max()
        wr = np.exp(nb)
        wr /= wr.sum()
        logits[r] = float(wr @ candidate_y[csel[o32]].astype(np.float64))
    return logits.astype(np.float32)


# revision 5
# speedup vs baseline: 2.9636x; 1.0842x over previous
"""MatchNet retrieval-KNN kernel for 8 Trainium2 NeuronCores.

Strategy (candidate-sharded, fp8 DoubleRow device screen + exact fp32 host
re-score):
  - Host: A = W^T W = V diag(lam) V^T.  Query rows q_i = sqrt(lam_i) (V^T x)_i,
    candidate rows r_i = sqrt(lam_i) (V^T c)_i, so s = (Ax).c = q.r.  The
    ranking-relevant bias g = -||c||_A^2/2 (mean-centered) is folded into the
    256-dim contraction by dropping the weakest eigendirection (lam_min ~ 0.11,
    error ~0.1 absolute, far below the fp8 screen noise) and using that slot as
    (alpha, g/alpha).  Both operands quantized to fp8 e4m3.
  - Device (per core, 12800-candidate shard): one DoubleRow fp8 matmul per
    [128q x 512c] PSUM tile (K=256 in one instruction, 0.5 cyc/row), then
    window-32 max-pooling of the scores:
      * "direct" chunks: DVE pool_max straight from PSUM fp32,
      * "drain" chunks: ACT copies PSUM->SBUF bf16, DVE pools the bf16 chunk,
    balancing the two engines.  Output: 400 bf16 window maxima per query per
    core - window position IS the candidate index, no index extraction needed.
  - Host: merge 8x400 window maxima per row, take the top W_SEL=64 windows,
    exactly re-score those 64*32 candidates in fp32 (s = (xA).c - cn2/2), take
    exact top-32, softmax(-dist), weighted-sum candidate_y.  Rows whose
    rank-32/33 gap is within fp32 rounding ambiguity are re-ranked with
    reference-style fp32 arithmetic.

Toolchain note: walrus here rejects >1 sync wait per instruction;
_legalize_waits() peels extra waits onto single-wait same-engine NoOps in
the BIR JSON (engines execute their stream in order, so blocking is
equivalent).
"""

import json
import os
import types

import ml_dtypes
import numpy as np

import concourse.bass as bass
import concourse.mybir as mybir
import concourse.tile as tile
from concourse.bass import ds
from concourse.bass_utils import run_bass_kernel_spmd

B, N, D_IN, DIM, NUMK = 1024, 100000, 256, 512, 32
TEMP = 1.0
NCORES = 8
NSHARD_REAL = N // NCORES  # 12500
NSHARD = 12800             # padded per-core candidate count
QT = B // 128              # 8 query tiles
NT = NSHARD // 512         # 25 candidate tiles per query tile
WIN = 32
NWIN = NSHARD // WIN       # 400 windows per row per core
CHUNK_T = 5                # tiles per chunk (for drain-path grouping)
NCHUNK = NT // CHUNK_T     # 5 chunks per qtile
DRAIN_CHUNKS = (0, 2, 4)   # chunks drained via ACT->bf16 (rest: DVE direct)
W_SEL = 64                 # host re-scores this many windows per row

F32 = mybir.dt.float32
BF16 = mybir.dt.bfloat16
FP8 = mybir.dt.float8e4
ACT_COPY = mybir.ActivationFunctionType.Copy
DR = mybir.MatmulPerfMode.DoubleRow


def _legalize_waits(nc):
    """Wrap nc.to_json_bytes so every instruction carries <=1 sync wait."""
    orig = nc.to_json_bytes

    def patched(self):
        m = json.loads(orig())
        ctr = 0
        for fn in m["functions"]:
            for blk in fn["blocks"]:
                out = []
                for inst in blk["instructions"]:
                    si = inst.get("sync_info")
                    waits = (si or {}).get("on_wait") or []
                    if len(waits) > 1:
                        for w in waits[:-1]:
                            ctr += 1
                            out.append({
                                "debug": inst.get("debug", 0),
                                "engine": inst["engine"],
                                "ins": [],
                                "name": f"I-nopw{ctr}",
                                "opcode": "NoOp",
                                "outs": [],
                                "sync_info": {"on_wait": [w],
                                              "on_update": []},
                            })
                        si["on_wait"] = waits[-1:]
                    out.append(inst)
                blk["instructions"] = out
        return json.dumps(m).encode()

    nc.to_json_bytes = types.MethodType(patched, nc)
    return nc


def _build_bass():
    nc = bass.Bass()
    xa_d = nc.dram_tensor("xa", [D_IN, B], FP8, kind="ExternalInput")
    cxt_d = nc.dram_tensor("cxt", [D_IN, NSHARD], FP8, kind="ExternalInput")
    oval_d = nc.dram_tensor("out_val", [B, NWIN], BF16,
                            kind="ExternalOutput")
    oval_r = oval_d.rearrange("(q p) w -> p q w", p=128)

    with (
        tile.TileContext(nc) as tc,
        tc.tile_pool(name="const", bufs=1) as constp,
        tc.tile_pool(name="s", bufs=3) as sp,
        tc.tile_pool(name="pp", bufs=3, space="PSUM") as ppp,
        tc.tile_pool(name="ps1", bufs=2, space="PSUM") as ps1p,
    ):
        xa_sb = constp.tile([128, 2, B], FP8)
        nc.sync.dma_start(
            xa_sb, xa_d.rearrange("(ko ki) q -> ki ko q", ki=128))
        cx_sb = constp.tile([128, 2, NSHARD], FP8)
        NSLC = 5
        SLC = NSHARD // NSLC
        for s in range(NSLC):
            nc.sync.dma_start(
                cx_sb[:, :, ds(s * SLC, SLC)],
                cxt_d[:, ds(s * SLC, SLC)].rearrange(
                    "(ko ki) n -> ki ko n", ki=128))

        uval_all = constp.tile([128, QT, NWIN], BF16, name="uval_all")

        # Per qtile: 12 psum pairs (2 banks) + 1 single-bank tile.
        # Pairs 0..9: ACT drains to bf16 buffers of 4 tiles, DVE reduces
        # each buffer.  Pairs 10-11 + the single: DVE reduces fp32 PSUM
        # directly.  Balances ACT ~80us vs DVE ~85us per core.
        AP_PAIRS = 10           # ACT-drained pairs per qtile

        def mm(ps, j, q, nt):
            nc.tensor.matmul(
                ps[:, j, :, :],
                xa_sb[:, :, ds(q * 128, 128)],
                cx_sb[:, :, ds(nt * 512, 512)],
                start=True, stop=True, perf_mode=DR)

        for q in range(QT):
            s_sb = None
            for p in range(AP_PAIRS):
                ps = ppp.tile([128, 2, 16, WIN], F32)
                mm(ps, 0, q, 2 * p)
                mm(ps, 1, q, 2 * p + 1)
                if p % 2 == 0:
                    s_sb = sp.tile([128, 4, 16, WIN], BF16, name="s_sb")
                nc.scalar.activation(
                    s_sb[:, ds((p % 2) * 2, 2), :, :], ps, ACT_COPY)
                if p % 2 == 1:
                    nc.vector.tensor_reduce(
                        uval_all[:, q, ds((p - 1) * 32, 64)], s_sb,
                        axis=mybir.AxisListType.X, op=mybir.AluOpType.max)
            for p in range(AP_PAIRS, 12):
                ps = ppp.tile([128, 2, 16, WIN], F32)
                mm(ps, 0, q, 2 * p)
                mm(ps, 1, q, 2 * p + 1)
                nc.vector.tensor_reduce(
                    uval_all[:, q, ds(p * 32, 32)], ps,
                    axis=mybir.AxisListType.X, op=mybir.AluOpType.max)
            ps = ps1p.tile([128, 16, WIN], F32)
            nc.tensor.matmul(
                ps,
                xa_sb[:, :, ds(q * 128, 128)],
                cx_sb[:, :, ds(24 * 512, 512)],
                start=True, stop=True, perf_mode=DR)
            nc.vector.tensor_reduce(
                uval_all[:, q, ds(384, 16)], ps,
                axis=mybir.AxisListType.X, op=mybir.AluOpType.max)
            nc.sync.dma_start(oval_r[:, q, :], uval_all[:, q, :])
    return _legalize_waits(nc)


_NC_CACHE = {}


def kernel(x, candidate_x, candidate_y, W, b, context_size, is_train):
    x = np.asarray(x, dtype=np.float32)
    candidate_x = np.asarray(candidate_x, dtype=np.float32)
    candidate_y = np.asarray(candidate_y, dtype=np.float32)
    W = np.asarray(W, dtype=np.float32)
    b = np.asarray(b, dtype=np.float32)

    A = (W.T @ W).astype(np.float64)           # [256, 256]
    lam, V = np.linalg.eigh(A)                 # ascending eigenvalues
    sq = np.sqrt(lam[1:]).astype(np.float32)   # drop weakest eigendirection
    Vk = V[:, 1:].astype(np.float32)           # [256, 255]

    Qm = (Vk * sq).T @ x.T                     # [255, B]
    Rm = (Vk * sq).T @ candidate_x.T           # [255, N]
    p0 = candidate_x @ V[:, 0].astype(np.float32)
    cn2 = np.sum(Rm.astype(np.float64) ** 2, axis=0) \
        + lam[0] * p0.astype(np.float64) ** 2  # c^T A c
    g = (-0.5 * cn2).astype(np.float32)
    gc = g - g.mean()                          # centered: ranking unchanged
    alpha = float(2.0 ** round(np.log2(np.sqrt(gc.std()))))

    xa = np.empty((D_IN, B), dtype=np.float32)
    xa[0, :] = alpha
    xa[1:, :] = Qm
    xa_f8 = xa.astype(ml_dtypes.float8_e4m3)

    in_maps = []
    for c in range(NCORES):
        cxt = np.zeros((D_IN, NSHARD), dtype=np.float32)
        sl = slice(c * NSHARD_REAL, (c + 1) * NSHARD_REAL)
        cxt[0, :NSHARD_REAL] = gc[sl] / alpha
        cxt[0, NSHARD_REAL:] = -240.0          # pads lose every window
        cxt[1:, :NSHARD_REAL] = Rm[:, sl]
        in_maps.append({"xa": xa_f8,
                        "cxt": cxt.astype(ml_dtypes.float8_e4m3)})

    if "nc" not in _NC_CACHE:
        _NC_CACHE["nc"] = _build_bass()
    nc = _NC_CACHE["nc"]

    trace = bool(int(os.environ.get("KERNEL_TRACE", "0")))
    res = run_bass_kernel_spmd(nc, in_maps, core_ids=list(range(NCORES)),
                               trace=trace)
    if trace:
        print(f"HW exec time: {res.exec_time_ns} ns")
        print(f"mean exec time: {res.mean_exec_time_ns} ns")
        if res.instructions_and_trace is not None:
            print("trace:", res.instructions_and_trace[1])

    # ---- host: merge window maxima, exact fp32 re-score of top windows ----
    pooled = np.concatenate(
        [r["out_val"].astype(np.float32) for r in res.results], axis=1)
    # window id -> global candidate base
    wcore = np.arange(NCORES * NWIN) // NWIN
    wwin = np.arange(NCORES * NWIN) % NWIN
    wbase = wcore * NSHARD_REAL + wwin * WIN   # [3200]

    rows = np.arange(B)[:, None]
    selw = np.argpartition(-pooled, W_SEL, axis=1)[:, :W_SEL]  # [B, 64]
    cand = (wbase[selw][:, :, None]
            + np.arange(WIN)[None, None, :]).reshape(B, -1)    # [B, 2048]
    valid = cand < ((cand // NSHARD_REAL) * NSHARD_REAL + NSHARD_REAL)
    # windows at a shard tail may spill past the real candidates
    valid &= cand < N
    cand_c = np.minimum(cand, N - 1)

    xA = (A.astype(np.float32).T @ x.T).T      # [B, 256] = x A
    C_sel = candidate_x[cand_c]                # [B, 2048, 256]
    s_sel = np.einsum("rd,rkd->rk", xA, C_sel,
                      dtype=np.float32, casting="same_kind")
    s_sel = s_sel.astype(np.float64) - 0.5 * cn2[cand_c]
    s_sel[~valid] = -1e30

    ordK = np.argsort(-s_sel, axis=1, kind="stable")
    top = ordK[:, :NUMK]
    s_top = np.take_along_axis(s_sel, top, axis=1)
    cand_top = np.take_along_axis(cand_c, top, axis=1)

    xe = (x @ W.T + b).astype(np.float32)
    xn2 = np.sum(xe.astype(np.float64) ** 2, axis=1)
    const_q = x.astype(np.float64) @ (W.T @ b).astype(np.float64) \
        + 0.5 * float(b.astype(np.float64) @ b.astype(np.float64))

    d2 = xn2[:, None] - 2.0 * (s_top + const_q[:, None])
    d = np.sqrt(np.maximum(d2, 0.0)) / TEMP
    neg = -d
    neg -= neg.max(axis=1, keepdims=True)
    w = np.exp(neg)
    w /= w.sum(axis=1, keepdims=True)
    logits = np.sum(w * candidate_y[cand_top].astype(np.float64), axis=1)

    # Rows whose rank-32/33 gap is within fp32 rounding ambiguity: re-rank
    # with reference-style fp32 arithmetic so the boundary pick matches.
    gap = (s_sel[rows[:, 0], ordK[:, NUMK - 1]]
           - s_sel[rows[:, 0], ordK[:, NUMK]])
    for r in np.where(gap < 0.01)[0]:
        csel = cand_c[r][valid[r]]
        ce_sel = (candidate_x[csel] @ W.T + b).astype(np.float32)
        sq_r = (np.sum(xe[r] ** 2, dtype=np.float32)
                + np.sum(ce_sel ** 2, axis=1, dtype=np.float32)
                - 2.0 * (ce_sel @ xe[r]))
        d_r = np.sqrt(np.maximum(sq_r, 0.0)) / TEMP
        o32 = np.argsort(d_r, kind="stable")[:NUMK]
        nb = (-d_r[o32]).astype(np.float64)
        nb -= nb.max()
        wr = np.exp(nb)
        wr /= wr.sum()
        logits[r] = float(wr @ candidate_y[csel[o32]].astype(np.float64))
    return logits.astype(np.float32)


# revision 10
# speedup vs baseline: 3.2445x; 1.0948x over previous
"""MatchNet retrieval-KNN kernel for 8 Trainium2 NeuronCores.

Strategy (candidate-sharded, fp8 DoubleRow device screen + exact fp32 host
re-score):
  - Host: A = W^T W = V diag(lam) V^T.  Query rows q_i = sqrt(lam_i) (V^T x)_i,
    candidate rows r_i = sqrt(lam_i) (V^T c)_i, so s = (Ax).c = q.r.  The
    ranking-relevant bias g = -||c||_A^2/2 (mean-centered) is folded into the
    256-dim contraction by dropping the weakest eigendirection (lam_min ~ 0.11,
    error ~0.1 absolute, far below the fp8 screen noise) and using that slot as
    (alpha, g/alpha).  Both operands quantized to fp8 e4m3.
  - Device (per core, 12800-candidate shard): one DoubleRow fp8 matmul per
    [128q x 512c] PSUM tile (K=256 in one instruction, 0.5 cyc/row), then
    window-32 max-pooling of the scores:
      * "direct" chunks: DVE pool_max straight from PSUM fp32,
      * "drain" chunks: ACT copies PSUM->SBUF bf16, DVE pools the bf16 chunk,
    balancing the two engines.  Output: 400 bf16 window maxima per query per
    core - window position IS the candidate index, no index extraction needed.
  - Host: merge 8x400 window maxima per row, take the top W_SEL=64 windows,
    exactly re-score those 64*32 candidates in fp32 (s = (xA).c - cn2/2), take
    exact top-32, softmax(-dist), weighted-sum candidate_y.  Rows whose
    rank-32/33 gap is within fp32 rounding ambiguity are re-ranked with
    reference-style fp32 arithmetic.

Toolchain note: walrus here rejects >1 sync wait per instruction;
_legalize_waits() peels extra waits onto single-wait same-engine NoOps in
the BIR JSON (engines execute their stream in order, so blocking is
equivalent).
"""

import json
import os
import types
from concurrent.futures import ThreadPoolExecutor

import ml_dtypes
import numpy as np

import concourse.bass as bass
import concourse.mybir as mybir
import concourse.tile as tile
from concourse.bass import ds
from concourse.bass_utils import run_bass_kernel_spmd

B, N, D_IN, DIM, NUMK = 1024, 100000, 256, 512, 32
TEMP = 1.0
NCORES = 8
NSHARD_REAL = N // NCORES  # 12500
NSHARD = 12800             # padded per-core candidate count
QT = B // 128              # 8 query tiles
NT = NSHARD // 512         # 25 candidate tiles per query tile
WIN = 32
NWIN = NSHARD // WIN       # 400 windows per row per core
CHUNK_T = 5                # tiles per chunk (for drain-path grouping)
NCHUNK = NT // CHUNK_T     # 5 chunks per qtile
DRAIN_CHUNKS = (0, 2, 4)   # chunks drained via ACT->bf16 (rest: DVE direct)
W_SEL = 64                 # host re-scores this many windows per row

F32 = mybir.dt.float32
BF16 = mybir.dt.bfloat16
FP8 = mybir.dt.float8e4
ACT_COPY = mybir.ActivationFunctionType.Copy
DR = mybir.MatmulPerfMode.DoubleRow


def _legalize_waits(nc):
    """Wrap nc.to_json_bytes so every instruction carries <=1 sync wait."""
    orig = nc.to_json_bytes

    def patched(self):
        m = json.loads(orig())
        ctr = 0
        for fn in m["functions"]:
            for blk in fn["blocks"]:
                out = []
                for inst in blk["instructions"]:
                    si = inst.get("sync_info")
                    waits = (si or {}).get("on_wait") or []
                    if len(waits) > 1:
                        for w in waits[:-1]:
                            ctr += 1
                            out.append({
                                "debug": inst.get("debug", 0),
                                "engine": inst["engine"],
                                "ins": [],
                                "name": f"I-nopw{ctr}",
                                "opcode": "NoOp",
                                "outs": [],
                                "sync_info": {"on_wait": [w],
                                              "on_update": []},
                            })
                        si["on_wait"] = waits[-1:]
                    out.append(inst)
                blk["instructions"] = out
        return json.dumps(m).encode()

    nc.to_json_bytes = types.MethodType(patched, nc)
    return nc


def _build_bass():
    nc = bass.Bass()
    xa_d = nc.dram_tensor("xa", [D_IN, B], FP8, kind="ExternalInput")
    cxt_d = nc.dram_tensor("cxt", [D_IN, NSHARD], FP8, kind="ExternalInput")
    oval_d = nc.dram_tensor("out_val", [B, NWIN], BF16,
                            kind="ExternalOutput")
    oval_r = oval_d.rearrange("(q p) w -> p q w", p=128)

    with (
        tile.TileContext(nc) as tc,
        tc.tile_pool(name="const", bufs=1) as constp,
        tc.tile_pool(name="s", bufs=3) as sp,
        tc.tile_pool(name="t", bufs=3) as tp_,
        tc.tile_pool(name="pp", bufs=3, space="PSUM") as ppp,
        tc.tile_pool(name="ps1", bufs=2, space="PSUM") as ps1p,
    ):
        xa_sb = constp.tile([128, 2, B], FP8)
        nc.sync.dma_start(
            xa_sb, xa_d.rearrange("(ko ki) q -> ki ko q", ki=128))
        cx_sb = constp.tile([128, 2, NSHARD], FP8)
        NSLC = 5
        SLC = NSHARD // NSLC
        dma_eng = [nc.sync, nc.scalar, nc.gpsimd, nc.sync, nc.scalar]
        for s in range(NSLC):
            dma_eng[s].dma_start(
                cx_sb[:, :, ds(s * SLC, SLC)],
                cxt_d[:, ds(s * SLC, SLC)].rearrange(
                    "(ko ki) n -> ki ko n", ki=128))

        uval_all = constp.tile([128, QT, NWIN], BF16, name="uval_all")

        # Per qtile: 12 psum pairs (2 banks each) + 1 single-bank tile.
        # "Direct" pairs: DVE tensor_reduce straight off fp32 PSUM
        # (~1.19us/pair).  Drained pairs: ACT copies PSUM->bf16 into
        # 4-tile buffers (~1.06us/pair) which DVE reduces with a 5-pass
        # binary tensor_tensor-max chain (bf16 runs the DVE 2x mode,
        # ~1.33us per 64-window buffer vs 2.19us for tensor_reduce).
        # 2.5 direct pairs + the single per qtile balances ACT ~80us
        # against DVE ~80us per core.

        def mm(ps, j, q, nt):
            nc.tensor.matmul(
                ps[:, ds(j * 16, 16), :],
                xa_sb[:, :, ds(q * 128, 128)],
                cx_sb[:, :, ds(nt * 512, 512)],
                start=True, stop=True, perf_mode=DR)

        def chain_reduce(s_sb, nw, out_ap):
            """Binary TT-max chain [128, nw, 32] bf16 -> out_ap [128, nw]."""
            cur = s_sb
            width = WIN
            while width > 2:
                nxt = tp_.tile([128, 64, width // 2], BF16,
                               name=f"t{width}")
                nc.vector.tensor_tensor(
                    nxt[:, 0:nw, :], cur[:, 0:nw, ds(0, width // 2)],
                    cur[:, 0:nw, ds(width // 2, width // 2)],
                    op=mybir.AluOpType.max)
                cur = nxt
                width //= 2
            nc.vector.tensor_tensor(
                out_ap, cur[:, 0:nw, ds(0, 1)], cur[:, 0:nw, ds(1, 1)],
                op=mybir.AluOpType.max)

        for q in range(QT):
            ndir = 3 if q % 2 == 0 else 2     # direct pairs this qtile
            ndrain = 12 - ndir
            # drained pairs first: 2 pairs -> one 4-tile buffer
            s_sb = None
            got = 0
            for p in range(ndrain):
                ps = ppp.tile([128, 32, WIN], F32)
                mm(ps, 0, q, 2 * p)
                mm(ps, 1, q, 2 * p + 1)
                if got == 0:
                    s_sb = sp.tile([128, 64, WIN], BF16, name="s_sb")
                nc.scalar.activation(
                    s_sb[:, ds(got * 32, 32), :], ps, ACT_COPY)
                got += 1
                if got == 2 or p == ndrain - 1:
                    nw = got * 32
                    w0 = (p + 1 - got) * 32
                    chain_reduce(s_sb, nw, uval_all[:, q, ds(w0, nw)])
                    got = 0
            for p in range(ndrain, 12):
                ps = ppp.tile([128, 32, WIN], F32)
                mm(ps, 0, q, 2 * p)
                mm(ps, 1, q, 2 * p + 1)
                nc.vector.tensor_reduce(
                    uval_all[:, q, ds(p * 32, 32)], ps,
                    axis=mybir.AxisListType.X, op=mybir.AluOpType.max)
            ps = ps1p.tile([128, 16, WIN], F32)
            nc.tensor.matmul(
                ps,
                xa_sb[:, :, ds(q * 128, 128)],
                cx_sb[:, :, ds(24 * 512, 512)],
                start=True, stop=True, perf_mode=DR)
            nc.vector.tensor_reduce(
                uval_all[:, q, ds(384, 16)], ps,
                axis=mybir.AxisListType.X, op=mybir.AluOpType.max)
            nc.gpsimd.dma_start(oval_r[:, q, :], uval_all[:, q, :])
    return _legalize_waits(nc)


_NC_CACHE = {}


def kernel(x, candidate_x, candidate_y, W, b, context_size, is_train):
    x = np.asarray(x, dtype=np.float32)
    candidate_x = np.asarray(candidate_x, dtype=np.float32)
    candidate_y = np.asarray(candidate_y, dtype=np.float32)
    W = np.asarray(W, dtype=np.float32)
    b = np.asarray(b, dtype=np.float32)

    A = (W.T @ W).astype(np.float64)           # [256, 256]
    lam, V = np.linalg.eigh(A)                 # ascending eigenvalues
    sq = np.sqrt(lam[1:]).astype(np.float32)   # drop weakest eigendirection
    Vk = V[:, 1:].astype(np.float32)           # [256, 255]

    Qm = (Vk * sq).T @ x.T                     # [255, B]
    Rm = (Vk * sq).T @ candidate_x.T           # [255, N]
    p0 = candidate_x @ V[:, 0].astype(np.float32)
    cn2 = np.sum(Rm.astype(np.float64) ** 2, axis=0) \
        + lam[0] * p0.astype(np.float64) ** 2  # c^T A c
    g = (-0.5 * cn2).astype(np.float32)
    gc = g - g.mean()                          # centered: ranking unchanged
    alpha = float(2.0 ** round(np.log2(np.sqrt(gc.std()))))

    xa = np.empty((D_IN, B), dtype=np.float32)
    xa[0, :] = alpha
    xa[1:, :] = Qm
    xa_f8 = xa.astype(ml_dtypes.float8_e4m3)

    in_maps = []
    for c in range(NCORES):
        cxt = np.zeros((D_IN, NSHARD), dtype=np.float32)
        sl = slice(c * NSHARD_REAL, (c + 1) * NSHARD_REAL)
        cxt[0, :NSHARD_REAL] = gc[sl] / alpha
        cxt[0, NSHARD_REAL:] = -240.0          # pads lose every window
        cxt[1:, :NSHARD_REAL] = Rm[:, sl]
        in_maps.append({"xa": xa_f8,
                        "cxt": cxt.astype(ml_dtypes.float8_e4m3)})

    if "nc" not in _NC_CACHE:
        _NC_CACHE["nc"] = _build_bass()
    nc = _NC_CACHE["nc"]

    trace = bool(int(os.environ.get("KERNEL_TRACE", "0")))
    res = run_bass_kernel_spmd(nc, in_maps, core_ids=list(range(NCORES)),
                               trace=trace)
    if trace:
        print(f"HW exec time: {res.exec_time_ns} ns")
        print(f"mean exec time: {res.mean_exec_time_ns} ns")
        if res.instructions_and_trace is not None:
            print("trace:", res.instructions_and_trace[1])

    # ---- host: merge window maxima, exact fp32 re-score of top windows ----
    pooled = np.concatenate(
        [r["out_val"].astype(np.float32) for r in res.results], axis=1)
    # window id -> global candidate base
    wcore = np.arange(NCORES * NWIN) // NWIN
    wwin = np.arange(NCORES * NWIN) % NWIN
    wbase = wcore * NSHARD_REAL + wwin * WIN   # [3200]

    rows = np.arange(B)[:, None]
    selw = np.argpartition(-pooled, W_SEL, axis=1)[:, :W_SEL]  # [B, 64]
    cand = (wbase[selw][:, :, None]
            + np.arange(WIN)[None, None, :]).reshape(B, -1)    # [B, 2048]
    valid = cand < ((cand // NSHARD_REAL) * NSHARD_REAL + NSHARD_REAL)
    # windows at a shard tail may spill past the real candidates
    valid &= cand < N
    cand_c = np.minimum(cand, N - 1)

    xA = (A.astype(np.float32).T @ x.T).T      # [B, 256] = x A
    s_sel32 = np.empty((B, cand_c.shape[1]), np.float32)

    def _rescore(r0, r1):
        C_sel = candidate_x[cand_c[r0:r1]]     # [rows, 2048, 256]
        s_sel32[r0:r1] = np.matmul(
            C_sel, xA[r0:r1, :, None])[:, :, 0]

    with ThreadPoolExecutor(8) as ex:
        step = B // 8
        list(ex.map(lambda i: _rescore(i * step, (i + 1) * step), range(8)))
    s_sel = s_sel32.astype(np.float64) - 0.5 * cn2[cand_c]
    s_sel[~valid] = -1e30

    ordK = np.argsort(-s_sel, axis=1, kind="stable")
    top = ordK[:, :NUMK]
    s_top = np.take_along_axis(s_sel, top, axis=1)
    cand_top = np.take_along_axis(cand_c, top, axis=1)

    xe = (x @ W.T + b).astype(np.float32)
    xn2 = np.sum(xe.astype(np.float64) ** 2, axis=1)
    const_q = x.astype(np.float64) @ (W.T @ b).astype(np.float64) \
        + 0.5 * float(b.astype(np.float64) @ b.astype(np.float64))

    d2 = xn2[:, None] - 2.0 * (s_top + const_q[:, None])
    d = np.sqrt(np.maximum(d2, 0.0)) / TEMP
    neg = -d
    neg -= neg.max(axis=1, keepdims=True)
    w = np.exp(neg)
    w /= w.sum(axis=1, keepdims=True)
    logits = np.sum(w * candidate_y[cand_top].astype(np.float64), axis=1)

    # Rows whose rank-32/33 gap is within fp32 rounding ambiguity: re-rank
    # with reference-style fp32 arithmetic so the boundary pick matches.
    gap = (s_sel[rows[:, 0], ordK[:, NUMK - 1]]
           - s_sel[rows[:, 0], ordK[:, NUMK]])
    for r in np.where(gap < 0.01)[0]:
        csel = cand_c[r][valid[r]]
        ce_sel = (candidate_x[csel] @ W.T + b).astype(np.float32)
        sq_r = (np.sum(xe[r] ** 2, dtype=np.float32)
                + np.sum(ce_sel ** 2, axis=1, dtype=np.float32)
                - 2.0 * (ce_sel @ xe[r]))
        d_r = np.sqrt(np.maximum(sq_r, 0.0)) / TEMP
        o32 = np.argsort(d_r, kind="stable")[:NUMK]
        nb = (-d_r[o32]).astype(np.float64)
        nb -= nb.max()
        wr = np.exp(nb)
        wr /= wr.sum()
        logits[r] = float(wr @ candidate_y[csel[o32]].astype(np.float64))
    return logits.astype(np.float32)


# revision 15
# speedup vs baseline: 3.5504x; 1.0943x over previous
"""MatchNet retrieval-KNN kernel for 8 Trainium2 NeuronCores.

Strategy (candidate-sharded, fp8 DoubleRow device screen + exact fp32 host
re-score):
  - Host: A = W^T W = V diag(lam) V^T.  Query rows q_i = sqrt(lam_i) (V^T x)_i,
    candidate rows r_i = sqrt(lam_i) (V^T c)_i, so s = (Ax).c = q.r.  The
    ranking-relevant bias g = -||c||_A^2/2 (mean-centered) is folded into the
    256-dim contraction by dropping the weakest eigendirection (lam_min ~ 0.11,
    error ~0.1 absolute, far below the fp8 screen noise) and using that slot as
    (alpha, g/alpha).  Both operands quantized to fp8 e4m3.
  - Device (per core, 12800-candidate shard): one DoubleRow fp8 matmul per
    [128q x 512c] PSUM tile (K=256 in one instruction, 0.5 cyc/row), then
    window-32 max-pooling of the scores:
      * "direct" chunks: DVE pool_max straight from PSUM fp32,
      * "drain" chunks: ACT copies PSUM->SBUF bf16, DVE pools the bf16 chunk,
    balancing the two engines.  Output: 400 bf16 window maxima per query per
    core - window position IS the candidate index, no index extraction needed.
  - Host: merge 8x400 window maxima per row, take the top W_SEL=64 windows,
    exactly re-score those 64*32 candidates in fp32 (s = (xA).c - cn2/2), take
    exact top-32, softmax(-dist), weighted-sum candidate_y.  Rows whose
    rank-32/33 gap is within fp32 rounding ambiguity are re-ranked with
    reference-style fp32 arithmetic.

Toolchain note: walrus here rejects >1 sync wait per instruction;
_legalize_waits() peels extra waits onto single-wait same-engine NoOps in
the BIR JSON (engines execute their stream in order, so blocking is
equivalent).
"""

import json
import os
import types
from concurrent.futures import ThreadPoolExecutor

import ml_dtypes
import numpy as np

import concourse.bass as bass
import concourse.mybir as mybir
import concourse.tile as tile
from concourse.bass import ds
from concourse.bass_utils import run_bass_kernel_spmd

B, N, D_IN, DIM, NUMK = 1024, 100000, 256, 512, 32
TEMP = 1.0
NCORES = 8
NSHARD_REAL = N // NCORES  # 12500
NSHARD = 12800             # padded per-core candidate count
QT = B // 128              # 8 query tiles
NT = NSHARD // 512         # 25 candidate tiles per query tile
WIN = 32
NWIN = NSHARD // WIN       # 400 windows per row per core
CHUNK_T = 5                # tiles per chunk (for drain-path grouping)
NCHUNK = NT // CHUNK_T     # 5 chunks per qtile
DRAIN_CHUNKS = (0, 2, 4)   # chunks drained via ACT->bf16 (rest: DVE direct)
W_SEL = 64                 # host re-scores this many windows per row

F32 = mybir.dt.float32
BF16 = mybir.dt.bfloat16
FP8 = mybir.dt.float8e4
ACT_COPY = mybir.ActivationFunctionType.Copy
DR = mybir.MatmulPerfMode.DoubleRow


def _legalize_waits(nc):
    """Wrap nc.to_json_bytes so every instruction carries <=1 sync wait."""
    orig = nc.to_json_bytes

    def patched(self):
        m = json.loads(orig())
        ctr = 0
        for fn in m["functions"]:
            for blk in fn["blocks"]:
                out = []
                for inst in blk["instructions"]:
                    si = inst.get("sync_info")
                    waits = (si or {}).get("on_wait") or []
                    if len(waits) > 1:
                        for w in waits[:-1]:
                            ctr += 1
                            out.append({
                                "debug": inst.get("debug", 0),
                                "engine": inst["engine"],
                                "ins": [],
                                "name": f"I-nopw{ctr}",
                                "opcode": "NoOp",
                                "outs": [],
                                "sync_info": {"on_wait": [w],
                                              "on_update": []},
                            })
                        si["on_wait"] = waits[-1:]
                    out.append(inst)
                blk["instructions"] = out
        return json.dumps(m).encode()

    nc.to_json_bytes = types.MethodType(patched, nc)
    return nc


def _build_bass():
    nc = bass.Bass()
    xa_d = nc.dram_tensor("xa", [D_IN, B], FP8, kind="ExternalInput")
    cxt_d = nc.dram_tensor("cxt", [D_IN, NSHARD], FP8, kind="ExternalInput")
    oval_d = nc.dram_tensor("out_val", [128, QT * NWIN], BF16,
                            kind="ExternalOutput")
    oval_f = oval_d[:, :]

    with (
        tile.TileContext(nc) as tc,
        tc.tile_pool(name="const", bufs=1) as constp,
        tc.tile_pool(name="s", bufs=3) as sp,
        tc.tile_pool(name="t", bufs=3) as tp_,
        tc.tile_pool(name="pp", bufs=2, space="PSUM") as ppp,
    ):
        xa_sb = constp.tile([128, 2, B], FP8)
        nc.sync.dma_start(
            xa_sb, xa_d.rearrange("(ko ki) q -> ki ko q", ki=128))
        cx_sb = constp.tile([128, 2, NSHARD], FP8)
        # ascending slice sizes, serialized on one DGE so the first
        # columns land early and compute starts ~9us in
        SLICES = (512, 1024, 2048, 4096, 5120)
        col = 0
        for w in SLICES:
            nc.sync.dma_start(
                cx_sb[:, :, ds(col, w)],
                cxt_d[:, ds(col, w)].rearrange(
                    "(ko ki) n -> ki ko n", ki=128))
            col += w

        # all 3200 window maxima, flat (q-major) so quads never split
        uval_all = constp.tile([128, QT * NWIN], BF16, name="uval_all")

        # 200 tiles as 50 uniform 4-bank PSUM quads (qtile boundaries fall
        # inside quads; each matmul picks its own lhsT slice).  Two quad
        # flavours, pattern AABAABAA (38 A / 12 B):
        #  A: ACT copies the quad to bf16 (~2.0us); two consecutive A-quads
        #     form one [128,128,32] buffer that DVE collapses with a 5-pass
        #     binary tensor_tensor-max chain (bf16 2x mode, ~2.6us).
        #  B: DVE tensor_tensor-max over the quad's fp32 window halves
        #     (dual-port read: 2048 elems at max_ap=1024, ~1.2us) + a
        #     4-pass bf16 chain (~0.9us).
        # Balances ACT ~77us vs DVE ~75us per core.

        def mm(ps, j, t):
            q, nt = t // NT, t % NT
            nc.tensor.matmul(
                ps[:, ds(j * 16, 16), :],
                xa_sb[:, :, ds(q * 128, 128)],
                cx_sb[:, :, ds(nt * 512, 512)],
                start=True, stop=True, perf_mode=DR)

        def chain(cur, nw, wflat, width):
            """Binary TT-max chain [128, nw, width] bf16 -> uval flat."""
            while width > 2:
                nxt = tp_.tile([128, 128, width // 2], BF16,
                               name=f"t{width}")
                nc.vector.tensor_tensor(
                    nxt[:, 0:nw, :], cur[:, 0:nw, ds(0, width // 2)],
                    cur[:, 0:nw, ds(width // 2, width // 2)],
                    op=mybir.AluOpType.max)
                cur = nxt
                width //= 2
            nc.vector.tensor_tensor(
                uval_all[:, ds(wflat, nw)], cur[:, 0:nw, ds(0, 1)],
                cur[:, 0:nw, ds(1, 1)], op=mybir.AluOpType.max)

        NQUAD = 200 // 4
        PERIOD = "AABAABAA"
        s_sb = None
        got = 0
        out_sent = 0
        for g in range(NQUAD):
            kind = PERIOD[g % len(PERIOD)]
            ps = ppp.tile([128, 64, WIN], F32)
            for j in range(4):
                mm(ps, j, 4 * g + j)
            if kind == "A":
                if got == 0:
                    s_sb = sp.tile([128, 128, WIN], BF16, name="s_sb")
                    sfirst = g
                nc.scalar.activation(
                    s_sb[:, ds(got * 64, 64), :], ps, ACT_COPY)
                got += 1
                if got == 2:
                    chain(s_sb, 128, sfirst * 64, WIN)
                    got = 0
            else:
                nc.vector.tensor_reduce(
                    uval_all[:, ds(g * 64, 64)], ps,
                    axis=mybir.AxisListType.X, op=mybir.AluOpType.max)
            # stream pooled output out as flat ranges complete
            if g in (25, 41) and got == 0:
                w_done = (g + 1) * 64
                nc.gpsimd.dma_start(
                    oval_f[:, ds(out_sent, w_done - out_sent)],
                    uval_all[:, ds(out_sent, w_done - out_sent)])
                out_sent = w_done
        if got == 1:          # dangling A-quad at the tail
            chain(s_sb, 64, sfirst * 64, WIN)
        if out_sent < QT * NWIN:
            nc.gpsimd.dma_start(
                oval_f[:, ds(out_sent, QT * NWIN - out_sent)],
                uval_all[:, ds(out_sent, QT * NWIN - out_sent)])
    return _legalize_waits(nc)


_NC_CACHE = {}


def kernel(x, candidate_x, candidate_y, W, b, context_size, is_train):
    x = np.asarray(x, dtype=np.float32)
    candidate_x = np.asarray(candidate_x, dtype=np.float32)
    candidate_y = np.asarray(candidate_y, dtype=np.float32)
    W = np.asarray(W, dtype=np.float32)
    b = np.asarray(b, dtype=np.float32)

    A = (W.T @ W).astype(np.float64)           # [256, 256]
    lam, V = np.linalg.eigh(A)                 # ascending eigenvalues
    sq = np.sqrt(lam[1:]).astype(np.float32)   # drop weakest eigendirection
    Vk = V[:, 1:].astype(np.float32)           # [256, 255]

    Qm = (Vk * sq).T @ x.T                     # [255, B]
    Rm = (Vk * sq).T @ candidate_x.T           # [255, N]
    p0 = candidate_x @ V[:, 0].astype(np.float32)
    cn2 = np.sum(Rm.astype(np.float64) ** 2, axis=0) \
        + lam[0] * p0.astype(np.float64) ** 2  # c^T A c
    g = (-0.5 * cn2).astype(np.float32)
    gc = g - g.mean()                          # centered: ranking unchanged
    alpha = float(2.0 ** round(np.log2(np.sqrt(gc.std()))))

    xa = np.empty((D_IN, B), dtype=np.float32)
    xa[0, :] = alpha
    xa[1:, :] = Qm
    xa_f8 = xa.astype(ml_dtypes.float8_e4m3)

    in_maps = []
    for c in range(NCORES):
        cxt = np.zeros((D_IN, NSHARD), dtype=np.float32)
        sl = slice(c * NSHARD_REAL, (c + 1) * NSHARD_REAL)
        cxt[0, :NSHARD_REAL] = gc[sl] / alpha
        cxt[0, NSHARD_REAL:] = -240.0          # pads lose every window
        cxt[1:, :NSHARD_REAL] = Rm[:, sl]
        in_maps.append({"xa": xa_f8,
                        "cxt": cxt.astype(ml_dtypes.float8_e4m3)})

    if "nc" not in _NC_CACHE:
        _NC_CACHE["nc"] = _build_bass()
    nc = _NC_CACHE["nc"]

    trace = bool(int(os.environ.get("KERNEL_TRACE", "0")))
    res = run_bass_kernel_spmd(nc, in_maps, core_ids=list(range(NCORES)),
                               trace=trace)
    if trace:
        print(f"HW exec time: {res.exec_time_ns} ns")
        print(f"mean exec time: {res.mean_exec_time_ns} ns")
        if res.instructions_and_trace is not None:
            print("trace:", res.instructions_and_trace[1])

    # ---- host: merge window maxima, exact fp32 re-score of top windows ----
    # device layout [128p, q*400+w] -> [1024 rows, 400 windows]
    pooled = np.concatenate(
        [r["out_val"].astype(np.float32).reshape(128, QT, NWIN)
         .transpose(1, 0, 2).reshape(B, NWIN)
         for r in res.results], axis=1)
    # window id -> global candidate base
    wcore = np.arange(NCORES * NWIN) // NWIN
    wwin = np.arange(NCORES * NWIN) % NWIN
    wbase = wcore * NSHARD_REAL + wwin * WIN   # [3200]

    rows = np.arange(B)[:, None]
    selw = np.argpartition(-pooled, W_SEL, axis=1)[:, :W_SEL]  # [B, 64]
    cand = (wbase[selw][:, :, None]
            + np.arange(WIN)[None, None, :]).reshape(B, -1)    # [B, 2048]
    valid = cand < ((cand // NSHARD_REAL) * NSHARD_REAL + NSHARD_REAL)
    # windows at a shard tail may spill past the real candidates
    valid &= cand < N
    cand_c = np.minimum(cand, N - 1)

    xA = (A.astype(np.float32).T @ x.T).T      # [B, 256] = x A
    s_sel32 = np.empty((B, cand_c.shape[1]), np.float32)

    def _rescore(r0, r1):
        C_sel = candidate_x[cand_c[r0:r1]]     # [rows, 2048, 256]
        s_sel32[r0:r1] = np.matmul(
            C_sel, xA[r0:r1, :, None])[:, :, 0]

    with ThreadPoolExecutor(8) as ex:
        step = B // 8
        list(ex.map(lambda i: _rescore(i * step, (i + 1) * step), range(8)))
    s_sel = s_sel32.astype(np.float64) - 0.5 * cn2[cand_c]
    s_sel[~valid] = -1e30

    ordK = np.argsort(-s_sel, axis=1, kind="stable")
    top = ordK[:, :NUMK]
    s_top = np.take_along_axis(s_sel, top, axis=1)
    cand_top = np.take_along_axis(cand_c, top, axis=1)

    xe = (x @ W.T + b).astype(np.float32)
    xn2 = np.sum(xe.astype(np.float64) ** 2, axis=1)
    const_q = x.astype(np.float64) @ (W.T @ b).astype(np.float64) \
        + 0.5 * float(b.astype(np.float64) @ b.astype(np.float64))

    d2 = xn2[:, None] - 2.0 * (s_top + const_q[:, None])
    d = np.sqrt(np.maximum(d2, 0.0)) / TEMP
    neg = -d
    neg -= neg.max(axis=1, keepdims=True)
    w = np.exp(neg)
    w /= w.sum(axis=1, keepdims=True)
    logits = np.sum(w * candidate_y[cand_top].astype(np.float64), axis=1)

    # Rows whose rank-32/33 gap is within fp32 rounding ambiguity: re-rank
    # with reference-style fp32 arithmetic so the boundary pick matches.
    gap = (s_sel[rows[:, 0], ordK[:, NUMK - 1]]
           - s_sel[rows[:, 0], ordK[:, NUMK]])
    for r in np.where(gap < 0.01)[0]:
        csel = cand_c[r][valid[r]]
        ce_sel = (candidate_x[csel] @ W.T + b).astype(np.float32)
        sq_r = (np.sum(xe[r] ** 2, dtype=np.float32)
                + np.sum(ce_sel ** 2, axis=1, dtype=np.float32)
                - 2.0 * (ce_sel @ xe[r]))
        d_r = np.sqrt(np.maximum(sq_r, 0.0)) / TEMP
        o32 = np.argsort(d_r, kind="stable")[:NUMK]
        nb = (-d_r[o32]).astype(np.float64)
        nb -= nb.max()
        wr = np.exp(nb)
        wr /= wr.sum()
        logits[r] = float(wr @ candidate_y[csel[o32]].astype(np.float64))
    return logits.astype(np.float32)
